# revision 1
# baseline (speedup 1.0000x reference)
"""Trainium2 Bass kernel for nn_CSFM_86011015070100 (topk_masking).

Data-parallel over batch: core b handles batch element b (B == 8 == n_cores).

Pipeline (per core):
  L1a (device): channel-sum (PE ones-matmul) and channel-max (PE transpose +
       DVE reduce) over C=256 for rgb and ir  -> [2,16384] sums, [2,128,128] maxs
  host: 7x7 conv + bias + double sigmoid in float64 -> spatial attention sa
       (f64 on host keeps sim ordering bit-stable vs the f32 reference; the
       min adjacent gap between sim values is ~1e-8, too tight for LUT sigmoid)
  L1b (device): per-channel dot(sa, x_c) and sum(x_c^2) partial sums
       (8-px / 1024-px blocks, combined on host in f64 for exact ordering)
  host: sims, stable argsort, positive counts, global k, gather tables
  L2  (device): indirect-DMA channel gather of rgb/ir + add -> output
  host: fix up the single max-fused channel (when k_rgb != k_ir)
"""

import numpy as np
from contextlib import ExitStack

import concourse.bass as bass
import concourse.bacc as bacc
import concourse.tile as tile
from concourse import mybir
from concourse.bass_utils import run_bass_kernel_spmd
from concourse.masks import make_identity

F32 = mybir.dt.float32
I32 = mybir.dt.int32

B, C, H, W = 8, 256, 128, 128
HW = H * W          # 16384
NCORES = 8
CORE_IDS = list(range(NCORES))
PCHUNK = 2048       # pixels per streamed chunk (1 MiB tiles)
NCHUNK = HW // PCHUNK
GCHUNK = 2048       # pixels per gather chunk in L2
NGCH = HW // GCHUNK

_cache = {}

# test harness hooks: set TRACE=True to collect per-launch HW exec times
TRACE = False
LAST_EXEC_NS = []


def _run(nc, maps):
    try:
        r = run_bass_kernel_spmd(nc, maps, CORE_IDS, trace=TRACE)
    except Exception:
        # rare transient NRT "exec unit unrecoverable" on the shared device;
        # one retry has been observed to succeed
        import time

        time.sleep(2)
        r = run_bass_kernel_spmd(nc, maps, CORE_IDS, trace=TRACE)
    if r.exec_time_ns is not None:
        LAST_EXEC_NS.append(r.exec_time_ns)
    return r.results


# --------------------------------------------------------------------------
# L1a: channel sum + channel max maps
# --------------------------------------------------------------------------
def _build_l1a():
    nc = bacc.Bacc("TRN2", target_bir_lowering=False, debug=False)
    rgb = nc.dram_tensor("rgb", [C, HW], F32, kind="ExternalInput").ap()
    ir = nc.dram_tensor("ir", [C, HW], F32, kind="ExternalInput").ap()
    sums = nc.dram_tensor("sums", [2, HW], F32, kind="ExternalOutput").ap()
    maxs = nc.dram_tensor("maxs", [2, H, W], F32, kind="ExternalOutput").ap()

    with tile.TileContext(nc) as tc, ExitStack() as ctx:
        consts = ctx.enter_context(tc.tile_pool(name="consts", bufs=1))
        ld = ctx.enter_context(tc.tile_pool(name="ld", bufs=6))
        gmp = ctx.enter_context(tc.tile_pool(name="gmp", bufs=3))
        cmaxp = ctx.enter_context(tc.tile_pool(name="cmaxp", bufs=2))
        sumsp = ctx.enter_context(tc.tile_pool(name="sumsp", bufs=3))
        psum = ctx.enter_context(tc.tile_pool(name="psum", bufs=4, space="PSUM"))
        psum1 = ctx.enter_context(tc.tile_pool(name="psum1", bufs=3, space="PSUM"))

        ident = consts.tile([128, 128], F32)
        make_identity(nc, ident[:])
        ones = consts.tile([128, 1], F32)
        nc.vector.memset(ones[:], 1.0)

        for t, x in enumerate((rgb, ir)):
            cmax_wh = cmaxp.tile([128, 128], F32, tag="cmax")
            for ci in range(NCHUNK):
                sl = slice(ci * PCHUNK, (ci + 1) * PCHUNK)
                x0 = ld.tile([128, PCHUNK], F32, tag="x0")
                x1 = ld.tile([128, PCHUNK], F32, tag="x1")
                nc.sync.dma_start(out=x0[:], in_=x[0:128, sl])
                nc.sync.dma_start(out=x1[:], in_=x[128:256, sl])

                # channel sum over all 256 channels: pair-sum groups on the
                # (otherwise idle) GpSimd, then a single fp32 ones-matmul
                gsum = gmp.tile([128, PCHUNK], F32, tag="gsum")
                nc.gpsimd.tensor_tensor(out=gsum[:], in0=x0[:], in1=x1[:],
                                        op=mybir.AluOpType.add)
                sums_c = sumsp.tile([1, PCHUNK], F32, tag="sums")
                for n4 in range(PCHUNK // 512):
                    ps = psum1.tile([1, 512], F32, tag="ps")
                    nc.tensor.matmul(ps[:], ones[:],
                                     gsum[:, n4 * 512:(n4 + 1) * 512],
                                     start=True, stop=True)
                    nc.scalar.copy(
                        out=sums_c[0:1, n4 * 512:(n4 + 1) * 512], in_=ps[:])
                nc.scalar.dma_start(out=sums[t:t + 1, sl], in_=sums_c[:])

                # channel max: combine groups, transpose 128x128 blocks into
                # 4-block PSUM tiles, one DVE reduce per 4 blocks
                gm = gmp.tile([128, PCHUNK], F32, tag="gm")
                nc.vector.tensor_tensor(out=gm[:], in0=x0[:], in1=x1[:],
                                        op=mybir.AluOpType.max)
                for q in range(PCHUNK // 512):
                    pt = psum.tile([128, 4, 128], F32, tag="pt")
                    for b4 in range(4):
                        b = q * 4 + b4
                        nc.tensor.transpose(pt[:, b4], gm[:, b * 128:(b + 1) * 128],
                                            ident[:])
                    hcol = ci * (PCHUNK // 128) + q * 4
                    nc.vector.tensor_reduce(
                        out=cmax_wh[:, hcol:hcol + 4], in_=pt[:],
                        axis=mybir.AxisListType.X, op=mybir.AluOpType.max)
            nc.scalar.dma_start(out=maxs[t], in_=cmax_wh[:])

    nc.compile()
    return nc


# --------------------------------------------------------------------------
# L1b: per-channel dot(sa, x_c) and sum(x_c^2)
# --------------------------------------------------------------------------
DBLK = 8     # pixels per dot partial-sum block (fp32 error floor)
SBLK = 1024  # pixels per sum-of-squares partial block
NDP = HW // DBLK    # 2048 dot partials per channel
NSP = HW // SBLK    # 128 sq partials per channel


def _build_l1b():
    nc = bacc.Bacc("TRN2", target_bir_lowering=False, debug=False)
    rgb = nc.dram_tensor("rgb", [C, HW], F32, kind="ExternalInput").ap()
    ir = nc.dram_tensor("ir", [C, HW], F32, kind="ExternalInput").ap()
    sa = nc.dram_tensor("sa", [1, HW], F32, kind="ExternalInput").ap()
    # fine-grained partial sums; host combines in f64 to keep sim ordering
    # within ~5e-8 of exact (min adjacent sim gap is ~7e-7 on this scale)
    dparts = nc.dram_tensor("dparts", [2, 2, 128, NDP], F32,
                            kind="ExternalOutput").ap()
    sparts = nc.dram_tensor("sparts", [2, 2, 128, NSP], F32,
                            kind="ExternalOutput").ap()

    with tile.TileContext(nc) as tc, ExitStack() as ctx:
        consts = ctx.enter_context(tc.tile_pool(name="consts", bufs=1))
        ld = ctx.enter_context(tc.tile_pool(name="ld", bufs=3))
        sc1 = ctx.enter_context(tc.tile_pool(name="sc1", bufs=3))
        sc2 = ctx.enter_context(tc.tile_pool(name="sc2", bufs=2))
        dpp = ctx.enter_context(tc.tile_pool(name="dpp", bufs=1))
        spp = ctx.enter_context(tc.tile_pool(name="spp", bufs=1))

        sa128 = consts.tile([128, HW], F32)

        dps = {}
        sps = {}
        for t in range(2):
            for g in range(2):
                dps[t, g] = dpp.tile([128, NDP], F32, tag=f"dp{t}{g}",
                                     name=f"dp{t}{g}")
                sps[t, g] = spp.tile([128, NSP], F32, tag=f"sp{t}{g}",
                                     name=f"sp{t}{g}")

        # interleave rgb (DVE product) and ir (GpSimd product) per chunk so
        # both engines stay busy; sa broadcast slices land just-in-time
        for g in range(2):
            for ci in range(NCHUNK):
                sl = slice(ci * PCHUNK, (ci + 1) * PCHUNK)
                if g == 0:
                    sa_b = bass.AP(tensor=sa.tensor, offset=sa.offset + ci * PCHUNK,
                                   ap=[[0, 128], [1, PCHUNK]])
                    nc.sync.dma_start(out=sa128[:, sl], in_=sa_b)
                for t, x in enumerate((rgb, ir)):
                    xt = ld.tile([128, PCHUNK], F32, tag=f"xt{t}")
                    nc.sync.dma_start(out=xt[:], in_=x[g * 128:(g + 1) * 128, sl])
                    prod = sc1.tile([128, PCHUNK], F32, tag="prod")
                    # the DVE<->GpSimd shared port serializes concurrent
                    # 2-input ops, so run most products on GpSimd (DVE is
                    # busy with the reduces) and a minority on DVE
                    step = 2 * ci + t
                    eng = nc.vector if step % 3 == 0 else nc.gpsimd
                    eng.tensor_tensor(out=prod[:], in0=xt[:], in1=sa128[:, sl],
                                      op=mybir.AluOpType.mult)
                    nc.vector.tensor_reduce(
                        out=dps[t, g][:, ci * (PCHUNK // DBLK):(ci + 1) * (PCHUNK // DBLK)],
                        in_=prod[:].rearrange("p (s q) -> p s q", q=DBLK),
                        axis=mybir.AxisListType.X, op=mybir.AluOpType.add)
                    for si in range(PCHUNK // SBLK):
                        sq = sc2.tile([128, SBLK], F32, tag="sq")
                        spos = ci * (PCHUNK // SBLK) + si
                        nc.scalar.activation(
                            out=sq[:], in_=xt[:, si * SBLK:(si + 1) * SBLK],
                            func=mybir.ActivationFunctionType.Square,
                            accum_out=sps[t, g][:, spos:spos + 1])
        for t in range(2):
            for g in range(2):
                nc.scalar.dma_start(out=dparts[t, g], in_=dps[t, g][:])
                nc.scalar.dma_start(out=sparts[t, g], in_=sps[t, g][:])

    nc.compile()
    return nc


# --------------------------------------------------------------------------
# L2: gather channels of rgb/ir by index and add
# --------------------------------------------------------------------------
def _build_l2():
    nc = bacc.Bacc("TRN2", target_bir_lowering=False, debug=False,
                   num_swdge_queues=2)
    rgb = nc.dram_tensor("rgb", [C, HW], F32, kind="ExternalInput").ap()
    ir = nc.dram_tensor("ir", [C, HW], F32, kind="ExternalInput").ap()
    gidx = nc.dram_tensor("gidx", [2, C], I32, kind="ExternalInput").ap()
    out = nc.dram_tensor("out", [C, HW], F32, kind="ExternalOutput").ap()

    with tile.TileContext(nc) as tc, ExitStack() as ctx:
        idxp = ctx.enter_context(tc.tile_pool(name="idxp", bufs=1))
        rp = ctx.enter_context(tc.tile_pool(name="rp", bufs=6))
        ip = ctx.enter_context(tc.tile_pool(name="ip", bufs=6))
        op = ctx.enter_context(tc.tile_pool(name="op", bufs=6))

        for g in range(2):
            idr = idxp.tile([128, 1], I32, tag=f"idr{g}")
            idi = idxp.tile([128, 1], I32, tag=f"idi{g}")
            nc.sync.dma_start(out=idr[:], in_=gidx[0, g * 128:(g + 1) * 128])
            nc.sync.dma_start(out=idi[:], in_=gidx[1, g * 128:(g + 1) * 128])
            for ci in range(NGCH):
                sl = slice(ci * GCHUNK, (ci + 1) * GCHUNK)
                rt = rp.tile([128, GCHUNK], F32, tag="rt")
                it = ip.tile([128, GCHUNK], F32, tag="it")
                nc.gpsimd.indirect_dma_start(
                    out=rt[:], out_offset=None, in_=rgb,
                    in_offset=bass.IndirectOffsetOnAxis(ap=idr[:, 0:1], axis=0),
                    element_offset=ci * GCHUNK)
                inst = nc.gpsimd.indirect_dma_start(
                    out=it[:], out_offset=None, in_=ir,
                    in_offset=bass.IndirectOffsetOnAxis(ap=idi[:, 0:1], axis=0),
                    element_offset=ci * GCHUNK)
                inst.ins.queue = "qPoolDynamic1"  # second SWDGE ring
                ot = op.tile([128, GCHUNK], F32, tag="ot")
                nc.vector.tensor_tensor(out=ot[:], in0=rt[:], in1=it[:],
                                        op=mybir.AluOpType.add)
                nc.sync.dma_start(out=out[g * 128:(g + 1) * 128, sl], in_=ot[:])

    nc.compile()
    return nc


def _get(name, builder):
    if name not in _cache:
        _cache[name] = builder()
    return _cache[name]


# --------------------------------------------------------------------------
# host glue
# --------------------------------------------------------------------------
def _sigmoid(x):
    return np.where(x >= 0, 1.0 / (1.0 + np.exp(-x)), np.exp(x) / (1.0 + np.exp(x)))


def _host_sa(sums, maxs, conv_w, conv_b):
    """sums [2,16384] (h-major), maxs [2,128,128] ([w,h] layout) -> sa [16384] f32"""
    cw = conv_w.astype(np.float64)   # [1,2,7,7]
    planes = []
    for t in range(2):
        avg = (sums[t].astype(np.float64) / C).reshape(H, W)
        mx = maxs[t].astype(np.float64).T     # [w,h] -> [h,w]
        pad = np.zeros((2, H + 6, W + 6))
        pad[0, 3:-3, 3:-3] = avg
        pad[1, 3:-3, 3:-3] = mx
        conv = np.zeros((H, W))
        for c in range(2):
            for kh in range(7):
                for kw in range(7):
                    conv += cw[0, c, kh, kw] * pad[c, kh:kh + H, kw:kw + W]
        planes.append(conv)
    m = np.maximum(planes[0], planes[1]) + float(conv_b[0])
    sa = _sigmoid(_sigmoid(m))
    return sa.reshape(-1).astype(np.float32)


def kernel(rgb, ir, conv_w, conv_b):
    rgb = np.ascontiguousarray(rgb, dtype=np.float32)
    ir = np.ascontiguousarray(ir, dtype=np.float32)
    conv_w = np.asarray(conv_w, dtype=np.float32)
    conv_b = np.asarray(conv_b, dtype=np.float32)

    rgb2 = rgb.reshape(B, C, HW)
    ir2 = ir.reshape(B, C, HW)
    LAST_EXEC_NS.clear()

    # ---- L1a
    nc1a = _get("l1a", _build_l1a)
    maps1 = [{"rgb": rgb2[b], "ir": ir2[b]} for b in range(B)]
    res1 = _run(nc1a, maps1)

    # ---- host conv + sigmoids
    sa_rows = [_host_sa(res1[b]["sums"], res1[b]["maxs"], conv_w, conv_b)
               for b in range(B)]

    # ---- L1b
    nc1b = _get("l1b", _build_l1b)
    maps2 = [{"rgb": rgb2[b], "ir": ir2[b], "sa": sa_rows[b][None, :]}
             for b in range(B)]
    res2 = _run(nc1b, maps2)

    # ---- host: sims, orders, counts, tables (f64 combine of partials)
    orders = np.zeros((B, 2, C), np.int64)
    cnts = np.zeros((B, 2), np.int64)
    for b in range(B):
        dparts = res2[b]["dparts"].astype(np.float64)  # [2,2,128,NDP]
        sparts = res2[b]["sparts"].astype(np.float64)  # [2,2,128,NSP]
        for t in range(2):
            dot = np.concatenate([dparts[t, 0].sum(-1), dparts[t, 1].sum(-1)])
            sq = np.concatenate([sparts[t, 0].sum(-1), sparts[t, 1].sum(-1)])
            tv = dot / np.maximum(np.sqrt(sq), 1e-30)
            orders[b, t] = np.argsort(tv, kind="stable")
            cnts[b, t] = int((tv > 0).sum())
    k_rgb = int(cnts[:, 0].max())
    k_ir = int(cnts[:, 1].max())
    ch = np.arange(C)
    src_rgb = ch.copy()
    src_ir = ch.copy()
    if k_rgb < k_ir:
        src_rgb[ch > k_rgb] -= 1
    elif k_ir < k_rgb:
        src_ir[ch > k_ir] -= 1

    # ---- L2
    nc2 = _get("l2", _build_l2)
    gidxs = []
    for b in range(B):
        g_r = orders[b, 0][src_rgb]
        g_i = orders[b, 1][src_ir]
        gidxs.append(np.stack([g_r, g_i]).astype(np.int32))
    maps3 = [{"rgb": rgb2[b], "ir": ir2[b], "gidx": gidxs[b]} for b in range(B)]
    res3 = _run(nc2, maps3)
    out = np.stack([res3[b]["out"].reshape(C, H, W) for b in range(B)])

    # ---- host fixup of the max-fused channel
    if k_rgb != k_ir:
        kpos = min(k_rgb, k_ir)
        for b in range(B):
            maxfea = np.maximum(rgb2[b, orders[b, 0][0]], ir2[b, orders[b, 1][0]])
            if k_rgb < k_ir:
                other = ir2[b, gidxs[b][1][kpos]]
            else:
                other = rgb2[b, gidxs[b][0][kpos]]
            out[b, kpos] = (maxfea + other).reshape(H, W)

    return out

